# revision 36
# baseline (speedup 1.0000x reference)
"""Trainium2 Bass kernel for nn_DelayCell (LMU / Pade-delay recurrent cell).

Math: the reference cell is linear until the final tanh, and the encoder
matrix is constant (all entries equal), so per (batch, unit) the output is a
causal convolution of the input's feature-mean with a per-unit kernel
    w_i[j] = C_i^T M_i^j (g_i B),   M_i = I + g_i A,  g_i = 1/theta_i
followed by tanh.  W (units x T) is numerically low rank (<= 32 at 1e-6
relative), so  y[b,t,:] = tanh( P @ (Q-conv u)[t] )  with P: [units, R],
Q: [R, T].

Device mapping (per 128-step time chunk):
    E_d[k, r] = u[d*128 + r + k - 127]     (Hankel tiles of u)
    Z for chunks 4j..4j+3 are stacked on PSUM partitions as G_j [128,128]:
    G_j = sum_d  S_{4j-d} @ E_d            (S_p = 4 consecutive 32-rank
                                            blocks of the k-reversed Q bank,
                                            a 128-wide slice of a zero-padded
                                            SBUF tile -> full-width stationary)
    Y_m = tanh( Z_m^T @ P^T )              (decode, K=32 matmuls)

All matmul operands are bf16 (PSUM accumulation stays f32).  u is
transposed to time-major on the PE (the DMA-xbar transpose globally fences
the DMA rings; the DVE StreamTranspose wedges the exec unit on HW), cast to
bf16, then parked in DRAM TWICE (plain and shifted-by-one) so the Hankel
reads can use 4-byte-aligned strides: partitions 0-63 hold odd taps read
from u_pad, partitions 64-127 hold even taps read from the shifted copy,
with the Q bank's rows permuted host-side to match.  (A direct bf16 Hankel
read has 2-byte partition strides, which wedges the DMA engines.)

Engine/ring roles:
  scalar (ACT ring): x first-half issue, zsg PSUM->SBUF copies, tanh
  sync   (SP ring):  x second-half issue, per-group y stores
  vector:            feature-sum reduces (quarter-batch granularity), casts
  gpsimd (SWDGE):    qte/pt preloads, u park + Hankel reads
  tensor:            conv + decode matmuls + u transposes
Emission is u0,u1,c0,u2,c1,u3,c2,c3: the PE transpose of batch b+2 (gated
on batch b+2's x arrival) sits AFTER compute(b)'s matmuls in the tensor
stream, so it lands while compute(b+1)'s inputs are still in flight instead
of blocking batch 0's matmuls ~15us (the old emission had uchain(2) ahead
of compute(0)).  No other engine has cross-batch hazards: vector only does
front-pipeline work, scalar/sync only back-pipeline work.

Sharding: data-parallel over batch, 4 batches per core on 8 cores.
"""

import contextlib
import os

import numpy as np

import concourse.bass as bass
import concourse.bacc as bacc
import concourse.tile as tile
from concourse import mybir
from concourse.bass_utils import run_bass_kernel_spmd

F32 = mybir.dt.float32
BF16 = mybir.dt.bfloat16

UNITS, ORDER, DIM, BATCH, T = 256, 6, 256, 32, 2048
NCORES = 8
BPC = BATCH // NCORES          # batches per core
L = 128                        # time chunk
NCH = T // L                   # 16 chunks
RANK = 16                      # rank-16 truncation error (2e-3 frobenius on
                               # the kernel bank) is below the bf16 noise
GPC = L // RANK                # chunks per PSUM group (8)
NG = NCH // GPC                # groups per batch (2)
TPAD = T + L                   # zero-padded u length
QPAD = GPC - 1                 # zero RANK-col blocks left of the Q bank
QW = (QPAD + NCH) * RANK       # qte width (368)
NQ = 2                         # x dma_starts per batch-half (0.5 MB each)
QCH = NCH // (2 * NQ)          # chunks per x piece (4)
# partition p of the Hankel tile holds tap k = KPERM[p]
KPERM = [2 * p + 1 for p in range(64)] + [2 * p for p in range(64)]

_compiled = {}


def _host_weights(theta, AT, Bmat, decoders, encoders):
    """Build the rank-RANK factorization P, Q of the conv kernel bank W."""
    th = np.asarray(theta, np.float64).reshape(UNITS)
    A = np.asarray(AT, np.float64).T
    Bv = np.asarray(Bmat, np.float64).reshape(ORDER)
    dec = np.asarray(decoders, np.float64).reshape(UNITS, ORDER, UNITS)
    # per-unit decoder vector C_i (block-diagonal structure of `decoders`)
    Cm = np.stack([dec[i, :, i] for i in range(UNITS)])      # [UNITS, ORDER]
    e0 = float(np.asarray(encoders, np.float64)[0, 0])        # uniform encoder

    g = 1.0 / th
    M = np.eye(ORDER)[None] + g[:, None, None] * A[None]      # [UNITS, 6, 6]
    w = np.empty((UNITS, T))
    p = g[:, None] * Bv[None, :]                              # [UNITS, 6]
    for j in range(T):
        w[:, j] = np.einsum('uo,uo->u', Cm, p)
        p = np.einsum('upo,uo->up', M, p)
    w *= e0                                                   # fold in encoder scale

    U, s, Vt = np.linalg.svd(w, full_matrices=False)
    P = (U[:, :RANK] * s[:RANK]).astype(np.float32)           # [UNITS, RANK]
    Q = Vt[:RANK, :].astype(np.float32)                       # [RANK, T]
    return P, Q


def _build_program():
    nc = bacc.Bacc(None)
    x_in = nc.dram_tensor("x", [BPC, T, DIM], F32, kind="ExternalInput")
    qte_in = nc.dram_tensor("qte", [L, QW], BF16, kind="ExternalInput")
    # block-diag [[P^T, 0], [0, P^T]] tiled 4x on partitions (row-tiled
    # matmuls read the moving operand at their own base partition):
    # decodes two 16-rank chunks per matmul
    pt_in = nc.dram_tensor("pt", [4 * 2 * RANK, 2 * UNITS], BF16,
                           kind="ExternalInput")
    id_in = nc.dram_tensor("ident", [L, L], F32, kind="ExternalInput")
    # y leaves the device in bf16 (half the store traffic); host upcasts
    y_out = nc.dram_tensor("y", [BPC, T, UNITS], BF16, kind="ExternalOutput")

    with tile.TileContext(nc) as tc:
        ctx = contextlib.ExitStack()
        with ctx:
            singles = ctx.enter_context(tc.tile_pool(name="singles", bufs=1))
            xpool = ctx.enter_context(tc.tile_pool(name="xin", bufs=BPC))
            upool = ctx.enter_context(tc.tile_pool(name="usb", bufs=2))
            utpool = ctx.enter_context(tc.tile_pool(name="uts", bufs=2))
            dpool = ctx.enter_context(tc.tile_pool(name="dall", bufs=BPC))
            zpool = ctx.enter_context(tc.tile_pool(name="zs", bufs=2))
            ypool = ctx.enter_context(tc.tile_pool(name="ys", bufs=BPC))
            drampool = ctx.enter_context(
                tc.tile_pool(name="dram", bufs=BPC, space="DRAM"))
            pz = ctx.enter_context(
                tc.tile_pool(name="pz", bufs=2, space="PSUM"))
            py = ctx.enter_context(
                tc.tile_pool(name="py", bufs=1, space="PSUM"))
            pu = ctx.enter_context(
                tc.tile_pool(name="pu", bufs=1, space="PSUM"))

            # preloads ride SWDGE: both HWDGE rings belong to the x stream
            qts = singles.tile([L, QW], BF16)
            nc.gpsimd.dma_start(out=qts[:], in_=qte_in[:])
            pts = singles.tile([4 * 2 * RANK, 2 * UNITS], BF16)
            nc.gpsimd.dma_start(out=pts[:], in_=pt_in[:])
            idn = singles.tile([L, L], F32)
            nc.gpsimd.dma_start(out=idn[:], in_=id_in[:])

            # ---- x loads batch-major at quarter-batch (0.5 MB) granularity
            # so the reduces can track DMA arrival; half g=0 on the ACT
            # ring, g=1 on the SP ring (x has no input deps so the rings
            # saturate HBM back to back)
            xts = []
            for b in range(BPC):
                xts.append(xpool.tile([L, NCH * DIM], F32, name=f"xt{b}",
                                      tag="xt"))

            def load_x(b):
                for g in range(2):
                    eng = nc.scalar if g == 0 else nc.sync
                    for q in range(NQ):
                        c0 = (g * NQ + q) * QCH
                        base = x_in[b, c0 * L:(c0 + QCH) * L, :]
                        src = bass.AP(base.tensor, base.offset,
                                      [[DIM, L], [L * DIM, QCH], [1, DIM]])
                        eng.dma_start(
                            out=xts[b][:, c0 * DIM:(c0 + QCH) * DIM], in_=src)

            def uchain(b):
                """u[t] = sum_d x[b,t,d], PE-transposed to time-major, cast
                to bf16, parked in DRAM twice (shifted), read back as
                tap-permuted Hankel tiles with 4-byte-aligned strides.  All
                DMAs ride SWDGE.  The whole chain runs at HALF-batch
                granularity (8 chunks): conv group j of batch b only needs
                Hankel chunks 0..8j+7, so the group-0 matmuls start one
                half-chain earlier (each DRAM hop costs ~2.5us of
                completion latency; halving the chain hides one round)."""
                xt = xts[b]
                usb = upool.tile([L, NCH + 1], F32, name=f"usb{b}", tag="usb")
                nc.vector.memset(usb[:, 0:1], 0.0)
                u_pad = drampool.tile([TPAD], BF16, name=f"u_pad{b}",
                                      tag="u_pad")
                u_padB = drampool.tile([TPAD], BF16, name=f"u_padB{b}",
                                       tag="u_padB")
                dall = dpool.tile([L, NCH * L], BF16, name=f"dall{b}",
                                  tag="dall")
                HC = NCH // 2                     # chunks per half (8)
                for h in range(2):
                    for p in (2 * h, 2 * h + 1):
                        nc.vector.reduce_sum(
                            out=usb[:, 1 + QCH * p:1 + QCH * (p + 1)],
                            in_=xt[:, p * QCH * DIM:
                                   (p + 1) * QCH * DIM].rearrange(
                                "r (m d) -> r m d", m=QCH),
                            axis=mybir.AxisListType.X)
                    # uts_h rows hold u chunks 8h-1 .. 8h+7 (separate PSUM
                    # and SBUF tiles per half: engine ops need 32-aligned
                    # base partitions, so slicing one [17, L] tile at row 9
                    # is illegal)
                    r0, r1 = (0, HC + 1) if h == 0 else (HC + 1, NCH + 1)
                    nr = r1 - r0
                    ut_ps = pu.tile([nr, L], F32, name=f"utps{b}_{h}",
                                    tag=f"utps{h}")
                    nc.tensor.transpose(ut_ps[:], usb[:, r0:r1], idn[:])
                    uts = utpool.tile([nr, L], BF16, name=f"uts{b}_{h}",
                                      tag=f"uts{h}")
                    nc.vector.tensor_copy(uts[:], ut_ps[:])
                    # the whole u roundtrip rides SWDGE: a data-dependent
                    # DMA on an HWDGE ring blocks the issuing engine (at
                    # issue) and the ring sequencer (at completion), which
                    # measurably stalls the x stream -- both variants were
                    # tried and regressed
                    pad_eng = nc.gpsimd
                    padB_eng = nc.gpsimd
                    pad_eng.dma_start(
                        out=bass.AP(u_pad.tensor, u_pad.offset + r0 * L,
                                    [[L, nr], [1, L]]),
                        in_=uts[:])
                    # shifted copy u_padB[i] = u_pad[i+1] so even taps also
                    # read from 4-byte-aligned addresses
                    if h == 0:
                        padB_eng.dma_start(
                            out=bass.AP(u_padB.tensor, u_padB.offset,
                                        [[1, L - 1]]),
                            in_=uts[0:1, 1:L])
                        padB_eng.dma_start(
                            out=bass.AP(u_padB.tensor, u_padB.offset + L - 1,
                                        [[L, HC], [1, L]]),
                            in_=uts[1:HC + 1, :])
                    else:
                        padB_eng.dma_start(
                            out=bass.AP(u_padB.tensor,
                                        u_padB.offset + L - 1 + HC * L,
                                        [[L, HC], [1, L]]),
                            in_=uts[:])
                    # Hankel read for chunks 8h..8h+7, tap-permuted:
                    # partitions 0-63 odd taps, 64-127 even taps; all
                    # strides/starts 4-byte aligned.  Batch 3's reads ride
                    # the SP ring: every trace shows tanh3 gapping ~4-5us
                    # on dall3 (q0 runs at ~120GB/s behind y traffic by
                    # then, while the x stream is long gone so the
                    # data-dependent-DMA-on-HWDGE hazard is just a sub-us
                    # ring stall against one y piece)
                    read_eng = nc.sync if b == BPC - 1 else nc.gpsimd
                    c0 = HC * h * L
                    read_eng.dma_start(
                        out=dall[0:64, c0:c0 + HC * L],
                        in_=bass.AP(u_pad.tensor, u_pad.offset + 2 + c0,
                                    [[2, 64], [1, HC * L]]))
                    read_eng.dma_start(
                        out=dall[64:128, c0:c0 + HC * L],
                        in_=bass.AP(u_padB.tensor, u_padB.offset + c0,
                                    [[2, 64], [1, HC * L]]))
                return dall

            def compute(b, dall):
                """rank-R conv with full-width stationaries, decode, tanh,
                per-group y stores.  G_j holds Z for chunks GPC*j..+GPC-1
                stacked on partition blocks."""
                ysg = ypool.tile([L, NCH * UNITS], BF16, name=f"ysg{b}",
                                 tag="ysg")
                for j in range(NG):
                    gt = pz.tile([L, L], F32, name=f"gt{b}_{j}", tag="gt")
                    last = GPC * j + GPC - 1
                    for d in range(last + 1):
                        cs = (GPC * j - d + QPAD) * RANK
                        nc.tensor.matmul(
                            gt[:],
                            qts[:, cs:cs + GPC * RANK],
                            dall[:, d * L:(d + 1) * L],
                            start=(d == 0), stop=(d == last))
                    # one bf16 copy of the whole group (on the ACT engine --
                    # GpSimd can't read PSUM, and vector is busy with the
                    # front-pipeline reduces), then 4 paired decode matmuls:
                    # stationary = two 16-rank Z blocks (32-aligned
                    # partitions), moving = block-diag [P^T|P^T] (N=512),
                    # each into its OWN 1-bank PSUM tile (concurrent
                    # row-tiled matmuls must not share a bank)
                    zsg = zpool.tile([L, L], BF16, name=f"zsg{b}_{j}",
                                     tag="zsg")
                    nc.scalar.copy(zsg[:], gt[:])
                    for c in range(4):
                        yt = py.tile([L, 2 * UNITS], F32,
                                     name=f"yt{b}_{j}_{c}", tag=f"yt{c}")
                        nc.tensor.matmul(yt[:], zsg[32 * c:32 * (c + 1), :],
                                         pts[32 * c:32 * (c + 1), :],
                                         start=True, stop=True,
                                         tile_position=(32 * c, 0))
                        m0 = GPC * j + 2 * c
                        nc.scalar.activation(
                            out=ysg[:, m0 * UNITS:(m0 + 2) * UNITS],
                            in_=yt[:],
                            func=mybir.ActivationFunctionType.Tanh)
                    # per-group y stores split by group across the rings
                    # (j0 -> SP, j1 -> ACT): y production averages 176GB/s
                    # = one full ring, so a single ring has zero slack and
                    # any y backlog starves the later Hankel reads.  The
                    # ACT ring is free once x3 drains; its issue cost sits
                    # in scalar's natural tanh gaps.  The very last store
                    # is split across both rings to halve the tail drain.
                    if b == BPC - 1 and j == NG - 1:
                        half = GPC // 2
                        for s, eng in ((0, nc.sync), (1, nc.scalar)):
                            m0 = j * GPC + s * half
                            base = y_out[b, m0 * L:(m0 + half) * L, :]
                            dst = bass.AP(base.tensor, base.offset,
                                          [[UNITS, L], [L * UNITS, half],
                                           [1, UNITS]])
                            eng.dma_start(
                                out=dst,
                                in_=ysg[:, m0 * UNITS:(m0 + half) * UNITS])
                    else:
                        base = y_out[b, j * GPC * L:(j + 1) * GPC * L, :]
                        dst = bass.AP(base.tensor, base.offset,
                                      [[UNITS, L], [L * UNITS, GPC],
                                       [1, UNITS]])
                        eng = nc.sync if j == 0 else nc.scalar
                        eng.dma_start(
                            out=dst,
                            in_=ysg[:, j * GPC * UNITS:(j + 1) * GPC * UNITS])

            # x3 is throttled on dall0-h0: Tile schedules each engine's
            # queue dependency-driven (NOT in emission order), so the x
            # prefetch can only be throttled via real data deps -- tiny
            # dall-sourced WAW writes into corners of each of x3's four
            # load regions.  This carves an HBM hole at ~27-30us in which
            # batches 0/1's parks+Hankel reads drain at full rate instead
            # of being starved until the x stream ends (first conv matmul
            # ~35 -> ~29).  The release chain must ride idle resources:
            # the f32 staging copy goes on SCALAR (idle until zsg0, which
            # is gated on the same data anyway) and the WAW writes are
            # HWDGE ring ops on scalar/sync (rings are empty post-x; the
            # engine-block until dall0-h0 lands is free).  Earlier
            # variants that routed these through vector (busy with r2) or
            # the SWDGE queue (head-of-line blocks u2's parks) regressed.
            # x runs unthrottled: throttling x3 on dall0 (tiny dall-sourced
            # WAW writes into x3's tile corners) reliably pulls the first
            # tanh from ~36us to ~32.8us, but x3 then lands ~44.5us --
            # past batch 3's chain slack (x3 is needed by ~37-38) -- and
            # tanh3 gaps ~5us waiting for dall3, netting a loss (measured
            # 73.5-73.9 vs 72.3 unthrottled across three release-chain
            # variants).
            load_x(0), load_x(1), load_x(2)
            dalls = [uchain(0)]
            dalls.append(uchain(1))
            load_x(3)
            for b in range(BPC):
                compute(b, dalls[b])
                if b + 2 < BPC:
                    dalls.append(uchain(b + 2))
    nc.finalize()
    return nc


def kernel(inputs, x0, encoders, theta, decoders, AT, Bmat):
    P, Q = _host_weights(theta, AT, Bmat, decoders, encoders)
    # qt[k, n*RANK+rho] = Q[rho, n*L + (L-1-k)]  (k-reversed within each block
    # so the device can read Hankel tiles of u with positive strides)
    qt = np.ascontiguousarray(
        Q.reshape(RANK, NCH, L)[:, :, ::-1].transpose(2, 1, 0).reshape(
            L, NCH * RANK))
    qte = np.zeros((L, QW), np.float32)
    qte[:, QPAD * RANK:] = qt
    qte = qte[KPERM]              # match the Hankel tiles' tap permutation
    qte_bf = _to_bf16(qte)
    p2 = np.zeros((2 * RANK, 2 * UNITS), np.float32)
    p2[:RANK, :UNITS] = P.T
    p2[RANK:, UNITS:] = P.T
    pt_bf = _to_bf16(np.tile(p2, (4, 1)))
    ident = np.eye(L, dtype=np.float32)

    if "nc" not in _compiled:
        _compiled["nc"] = _build_program()
    nc = _compiled["nc"]

    x = np.ascontiguousarray(np.asarray(inputs, np.float32))
    in_maps = []
    for c in range(NCORES):
        in_maps.append({
            "x": x[c * BPC:(c + 1) * BPC],
            "qte": qte_bf, "pt": pt_bf, "ident": ident,
        })
    trace = bool(os.environ.get("BASS_TRACE"))
    res = run_bass_kernel_spmd(nc, in_maps, core_ids=list(range(NCORES)),
                               trace=trace)
    _compiled["last_results"] = res
    if res.exec_time_ns is not None:
        print(f"HW exec time: {res.exec_time_ns} ns")
    y = np.concatenate([np.asarray(r["y"], np.float32) for r in res.results],
                       axis=0)
    return y


def _to_bf16(a):
    import ml_dtypes
    return np.asarray(a, np.float32).astype(ml_dtypes.bfloat16)


# revision 47
# speedup vs baseline: 1.0170x; 1.0170x over previous
"""Trainium2 Bass kernel for nn_DelayCell (LMU / Pade-delay recurrent cell).

Math: the reference cell is linear until the final tanh, and the encoder
matrix is constant (all entries equal), so per (batch, unit) the output is a
causal convolution of the input's feature-mean with a per-unit kernel
    w_i[j] = C_i^T M_i^j (g_i B),   M_i = I + g_i A,  g_i = 1/theta_i
followed by tanh.  W (units x T) is numerically low rank (<= 32 at 1e-6
relative), so  y[b,t,:] = tanh( P @ (Q-conv u)[t] )  with P: [units, R],
Q: [R, T].

Device mapping (per 128-step time chunk):
    E_d[k, r] = u[d*128 + r + k - 127]     (Hankel tiles of u)
    Z for chunks 4j..4j+3 are stacked on PSUM partitions as G_j [128,128]:
    G_j = sum_d  S_{4j-d} @ E_d            (S_p = 4 consecutive 32-rank
                                            blocks of the k-reversed Q bank,
                                            a 128-wide slice of a zero-padded
                                            SBUF tile -> full-width stationary)
    Y_m = tanh( Z_m^T @ P^T )              (decode, K=32 matmuls)

All matmul operands are bf16 (PSUM accumulation stays f32).  u is
transposed to time-major on the PE (the DMA-xbar transpose globally fences
the DMA rings; the DVE StreamTranspose wedges the exec unit on HW), cast to
bf16, then parked in DRAM TWICE (plain and shifted-by-one) so the Hankel
reads can use 4-byte-aligned strides: partitions 0-63 hold odd taps read
from u_pad, partitions 64-127 hold even taps read from the shifted copy,
with the Q bank's rows permuted host-side to match.  (A direct bf16 Hankel
read has 2-byte partition strides, which wedges the DMA engines.)

Engine/ring roles:
  scalar (ACT ring): x first-half issue, zsg PSUM->SBUF copies, tanh
  sync   (SP ring):  x second-half issue, per-group y stores
  vector:            feature-sum reduces (quarter-batch granularity), casts
  gpsimd (SWDGE):    qte/pt preloads, u park + Hankel reads
  tensor:            conv + decode matmuls + u transposes
Emission is u0,u1,c0,u2,c1,u3,c2,c3: the PE transpose of batch b+2 (gated
on batch b+2's x arrival) sits AFTER compute(b)'s matmuls in the tensor
stream, so it lands while compute(b+1)'s inputs are still in flight instead
of blocking batch 0's matmuls ~15us (the old emission had uchain(2) ahead
of compute(0)).  No other engine has cross-batch hazards: vector only does
front-pipeline work, scalar/sync only back-pipeline work.

Sharding: data-parallel over batch, 4 batches per core on 8 cores.
"""

import contextlib
import os

import numpy as np

import concourse.bass as bass
import concourse.bacc as bacc
import concourse.tile as tile
from concourse import mybir
from concourse.bass_utils import run_bass_kernel_spmd

F32 = mybir.dt.float32
BF16 = mybir.dt.bfloat16

UNITS, ORDER, DIM, BATCH, T = 256, 6, 256, 32, 2048
NCORES = 8
BPC = BATCH // NCORES          # batches per core
L = 128                        # time chunk
NCH = T // L                   # 16 chunks
RANK = 16                      # rank-16 truncation error (2e-3 frobenius on
                               # the kernel bank) is below the bf16 noise
GPC = L // RANK                # chunks per PSUM group (8)
NG = NCH // GPC                # groups per batch (2)
TPAD = T + L                   # zero-padded u length
QPAD = GPC - 1                 # zero RANK-col blocks left of the Q bank
QW = (QPAD + NCH) * RANK       # qte width (368)
NQ = 2                         # x dma_starts per batch-half (0.5 MB each)
QCH = NCH // (2 * NQ)          # chunks per x piece (4)
# partition p of the Hankel tile holds tap k = KPERM[p]
KPERM = [2 * p + 1 for p in range(64)] + [2 * p for p in range(64)]

_compiled = {}


def _host_weights(theta, AT, Bmat, decoders, encoders):
    """Build the rank-RANK factorization P, Q of the conv kernel bank W."""
    th = np.asarray(theta, np.float64).reshape(UNITS)
    A = np.asarray(AT, np.float64).T
    Bv = np.asarray(Bmat, np.float64).reshape(ORDER)
    dec = np.asarray(decoders, np.float64).reshape(UNITS, ORDER, UNITS)
    # per-unit decoder vector C_i (block-diagonal structure of `decoders`)
    Cm = np.stack([dec[i, :, i] for i in range(UNITS)])      # [UNITS, ORDER]
    e0 = float(np.asarray(encoders, np.float64)[0, 0])        # uniform encoder

    g = 1.0 / th
    M = np.eye(ORDER)[None] + g[:, None, None] * A[None]      # [UNITS, 6, 6]
    w = np.empty((UNITS, T))
    p = g[:, None] * Bv[None, :]                              # [UNITS, 6]
    for j in range(T):
        w[:, j] = np.einsum('uo,uo->u', Cm, p)
        p = np.einsum('upo,uo->up', M, p)
    w *= e0                                                   # fold in encoder scale

    U, s, Vt = np.linalg.svd(w, full_matrices=False)
    P = (U[:, :RANK] * s[:RANK]).astype(np.float32)           # [UNITS, RANK]
    Q = Vt[:RANK, :].astype(np.float32)                       # [RANK, T]
    return P, Q


def _build_program():
    nc = bacc.Bacc(None)
    x_in = nc.dram_tensor("x", [BPC, T, DIM], F32, kind="ExternalInput")
    qte_in = nc.dram_tensor("qte", [L, QW], BF16, kind="ExternalInput")
    # block-diag [[P^T, 0], [0, P^T]] tiled 4x on partitions (row-tiled
    # matmuls read the moving operand at their own base partition):
    # decodes two 16-rank chunks per matmul
    pt_in = nc.dram_tensor("pt", [4 * 2 * RANK, 2 * UNITS], BF16,
                           kind="ExternalInput")
    id_in = nc.dram_tensor("ident", [L, L], F32, kind="ExternalInput")
    # y leaves the device in bf16 (half the store traffic); host upcasts
    y_out = nc.dram_tensor("y", [BPC, T, UNITS], BF16, kind="ExternalOutput")

    with tile.TileContext(nc) as tc:
        ctx = contextlib.ExitStack()
        with ctx:
            singles = ctx.enter_context(tc.tile_pool(name="singles", bufs=1))
            xpool = ctx.enter_context(tc.tile_pool(name="xin", bufs=BPC))
            upool = ctx.enter_context(tc.tile_pool(name="usb", bufs=2))
            utpool = ctx.enter_context(tc.tile_pool(name="uts", bufs=2))
            dpool = ctx.enter_context(tc.tile_pool(name="dall", bufs=BPC))
            zpool = ctx.enter_context(tc.tile_pool(name="zs", bufs=2))
            ypool = ctx.enter_context(tc.tile_pool(name="ys", bufs=2))
            drampool = ctx.enter_context(
                tc.tile_pool(name="dram", bufs=BPC, space="DRAM"))
            pz = ctx.enter_context(
                tc.tile_pool(name="pz", bufs=2, space="PSUM"))
            py = ctx.enter_context(
                tc.tile_pool(name="py", bufs=1, space="PSUM"))
            pu = ctx.enter_context(
                tc.tile_pool(name="pu", bufs=1, space="PSUM"))

            # preloads ride SWDGE: both HWDGE rings belong to the x stream
            qts = singles.tile([L, QW], BF16)
            nc.gpsimd.dma_start(out=qts[:], in_=qte_in[:])
            pts = singles.tile([4 * 2 * RANK, 2 * UNITS], BF16)
            nc.gpsimd.dma_start(out=pts[:], in_=pt_in[:])
            idn = singles.tile([L, L], F32)
            nc.gpsimd.dma_start(out=idn[:], in_=id_in[:])



            # ---- x loads batch-major at quarter-batch (0.5 MB) granularity
            # so the reduces can track DMA arrival; half g=0 on the ACT
            # ring, g=1 on the SP ring (x has no input deps so the rings
            # saturate HBM back to back)
            xts = []
            for b in range(BPC):
                xts.append(xpool.tile([L, NCH * DIM], F32, name=f"xt{b}",
                                      tag="xt"))

            def load_x(b):
                for g in range(2):
                    eng = nc.scalar if g == 0 else nc.sync
                    for q in range(NQ):
                        c0 = (g * NQ + q) * QCH
                        base = x_in[b, c0 * L:(c0 + QCH) * L, :]
                        src = bass.AP(base.tensor, base.offset,
                                      [[DIM, L], [L * DIM, QCH], [1, DIM]])
                        eng.dma_start(
                            out=xts[b][:, c0 * DIM:(c0 + QCH) * DIM], in_=src)

            def uchain(b):
                """u[t] = sum_d x[b,t,d], PE-transposed to time-major, cast
                to bf16, parked in DRAM twice (shifted), read back as
                tap-permuted Hankel tiles with 4-byte-aligned strides.  All
                DMAs ride SWDGE.  The whole chain runs at HALF-batch
                granularity (8 chunks): conv group j of batch b only needs
                Hankel chunks 0..8j+7, so the group-0 matmuls start one
                half-chain earlier (each DRAM hop costs ~2.5us of
                completion latency; halving the chain hides one round)."""
                xt = xts[b]
                usb = upool.tile([L, NCH + 1], F32, name=f"usb{b}", tag="usb")
                nc.vector.memset(usb[:, 0:1], 0.0)
                u_pad = drampool.tile([TPAD], BF16, name=f"u_pad{b}",
                                      tag="u_pad")
                u_padB = drampool.tile([TPAD], BF16, name=f"u_padB{b}",
                                       tag="u_padB")
                dall = dpool.tile([L, NCH * L], BF16, name=f"dall{b}",
                                  tag="dall")
                HC = NCH // 2                     # chunks per half (8)
                for h in range(2):
                    for p in (2 * h, 2 * h + 1):
                        nc.vector.reduce_sum(
                            out=usb[:, 1 + QCH * p:1 + QCH * (p + 1)],
                            in_=xt[:, p * QCH * DIM:
                                   (p + 1) * QCH * DIM].rearrange(
                                "r (m d) -> r m d", m=QCH),
                            axis=mybir.AxisListType.X)
                    # uts_h rows hold u chunks 8h-1 .. 8h+7 (separate PSUM
                    # and SBUF tiles per half: engine ops need 32-aligned
                    # base partitions, so slicing one [17, L] tile at row 9
                    # is illegal)
                    r0, r1 = (0, HC + 1) if h == 0 else (HC + 1, NCH + 1)
                    nr = r1 - r0
                    ut_ps = pu.tile([nr, L], F32, name=f"utps{b}_{h}",
                                    tag=f"utps{h}")
                    nc.tensor.transpose(ut_ps[:], usb[:, r0:r1], idn[:])
                    uts = utpool.tile([nr, L], BF16, name=f"uts{b}_{h}",
                                      tag=f"uts{h}")
                    nc.vector.tensor_copy(uts[:], ut_ps[:])
                    # the whole u roundtrip rides SWDGE: a data-dependent
                    # DMA on an HWDGE ring blocks the issuing engine (at
                    # issue) and the ring sequencer (at completion), which
                    # measurably stalls the x stream -- both variants were
                    # tried and regressed
                    nc.gpsimd.dma_start(
                        out=bass.AP(u_pad.tensor, u_pad.offset + r0 * L,
                                    [[L, nr], [1, L]]),
                        in_=uts[:])
                    # shifted copy u_padB[i] = u_pad[i+1] so even taps also
                    # read from 4-byte-aligned addresses
                    if h == 0:
                        nc.gpsimd.dma_start(
                            out=bass.AP(u_padB.tensor, u_padB.offset,
                                        [[1, L - 1]]),
                            in_=uts[0:1, 1:L])
                        nc.gpsimd.dma_start(
                            out=bass.AP(u_padB.tensor, u_padB.offset + L - 1,
                                        [[L, HC], [1, L]]),
                            in_=uts[1:HC + 1, :])
                    else:
                        nc.gpsimd.dma_start(
                            out=bass.AP(u_padB.tensor,
                                        u_padB.offset + L - 1 + HC * L,
                                        [[L, HC], [1, L]]),
                            in_=uts[:])
                    # Hankel read for chunks 8h..8h+7, tap-permuted:
                    # partitions 0-63 odd taps, 64-127 even taps; all
                    # strides/starts 4-byte aligned.  (Tried: batch 3's
                    # reads on the SP ring to dodge q0's ~120GB/s -- the
                    # sync engine-block plus ring stall snowballs the
                    # zero-slack y chain, 73.9us vs 72.7.)
                    read_eng = nc.gpsimd
                    c0 = HC * h * L
                    read_eng.dma_start(
                        out=dall[0:64, c0:c0 + HC * L],
                        in_=bass.AP(u_pad.tensor, u_pad.offset + 2 + c0,
                                    [[2, 64], [1, HC * L]]))
                    read_eng.dma_start(
                        out=dall[64:128, c0:c0 + HC * L],
                        in_=bass.AP(u_padB.tensor, u_padB.offset + c0,
                                    [[2, 64], [1, HC * L]]))
                return dall

            def compute(b, dall):
                """rank-R conv with full-width stationaries, decode, tanh,
                per-group y stores.  G_j holds Z for chunks GPC*j..+GPC-1
                stacked on partition blocks."""
                ysg = ypool.tile([L, NCH * UNITS], BF16, name=f"ysg{b}",
                                 tag="ysg")
                for j in range(NG):
                    gt = pz.tile([L, L], F32, name=f"gt{b}_{j}", tag="gt")
                    last = GPC * j + GPC - 1
                    for d in range(last + 1):
                        cs = (GPC * j - d + QPAD) * RANK
                        nc.tensor.matmul(
                            gt[:],
                            qts[:, cs:cs + GPC * RANK],
                            dall[:, d * L:(d + 1) * L],
                            start=(d == 0), stop=(d == last))
                    # one bf16 copy of the whole group (on the ACT engine --
                    # GpSimd can't read PSUM, and vector is busy with the
                    # front-pipeline reduces), then 4 paired decode matmuls:
                    # stationary = two 16-rank Z blocks (32-aligned
                    # partitions), moving = block-diag [P^T|P^T] (N=512),
                    # each into its OWN 1-bank PSUM tile (concurrent
                    # row-tiled matmuls must not share a bank)
                    zsg = zpool.tile([L, L], BF16, name=f"zsg{b}_{j}",
                                     tag="zsg")
                    nc.scalar.copy(zsg[:], gt[:])
                    for c in range(4):
                        yt = py.tile([L, 2 * UNITS], F32,
                                     name=f"yt{b}_{j}_{c}", tag=f"yt{c}")
                        nc.tensor.matmul(yt[:], zsg[32 * c:32 * (c + 1), :],
                                         pts[32 * c:32 * (c + 1), :],
                                         start=True, stop=True,
                                         tile_position=(32 * c, 0))
                        m0 = GPC * j + 2 * c
                        nc.scalar.activation(
                            out=ysg[:, m0 * UNITS:(m0 + 2) * UNITS],
                            in_=yt[:],
                            func=mybir.ActivationFunctionType.Tanh)
                    # per-group y stores split by group across the rings
                    # (j0 -> SP, j1 -> ACT): y production averages 176GB/s
                    # = one full ring, so a single ring has zero slack and
                    # any y backlog starves the later Hankel reads.  The
                    # ACT ring is free once x3 drains; its issue cost sits
                    # in scalar's natural tanh gaps.  The very last store
                    # is split across both rings to halve the tail drain.
                    if b == BPC - 1 and j == NG - 1:
                        half = GPC // 2
                        for s, eng in ((0, nc.sync), (1, nc.scalar)):
                            m0 = j * GPC + s * half
                            base = y_out[b, m0 * L:(m0 + half) * L, :]
                            dst = bass.AP(base.tensor, base.offset,
                                          [[UNITS, L], [L * UNITS, half],
                                           [1, UNITS]])
                            eng.dma_start(
                                out=dst,
                                in_=ysg[:, m0 * UNITS:(m0 + half) * UNITS])
                    else:
                        base = y_out[b, j * GPC * L:(j + 1) * GPC * L, :]
                        dst = bass.AP(base.tensor, base.offset,
                                      [[UNITS, L], [L * UNITS, GPC],
                                       [1, UNITS]])
                        nc.sync.dma_start(
                            out=dst,
                            in_=ysg[:, j * GPC * UNITS:(j + 1) * GPC * UNITS])

            # x3 is throttled on dall0-h0: Tile schedules each engine's
            # queue dependency-driven (NOT in emission order), so the x
            # prefetch can only be throttled via real data deps -- tiny
            # dall-sourced WAW writes into corners of each of x3's four
            # load regions.  This carves an HBM hole at ~27-30us in which
            # batches 0/1's parks+Hankel reads drain at full rate instead
            # of being starved until the x stream ends (first conv matmul
            # ~35 -> ~29).  The release chain must ride idle resources:
            # the f32 staging copy goes on SCALAR (idle until zsg0, which
            # is gated on the same data anyway) and the WAW writes are
            # HWDGE ring ops on scalar/sync (rings are empty post-x; the
            # engine-block until dall0-h0 lands is free).  Earlier
            # variants that routed these through vector (busy with r2) or
            # the SWDGE queue (head-of-line blocks u2's parks) regressed.
            # x runs unthrottled: throttling x3 on dall0 (tiny dall-sourced
            # WAW writes into x3's tile corners) reliably pulls the first
            # tanh from ~36us to ~32.8us, but x3 then lands ~44.5us --
            # past batch 3's chain slack (x3 is needed by ~37-38) -- and
            # tanh3 gaps ~5us waiting for dall3, netting a loss (measured
            # 73.5-73.9 vs 72.3 unthrottled across three release-chain
            # variants).
            load_x(0), load_x(1), load_x(2)
            dalls = [uchain(0)]
            dalls.append(uchain(1))
            load_x(3)
            for b in range(BPC):
                compute(b, dalls[b])
                if b + 2 < BPC:
                    dalls.append(uchain(b + 2))
    nc.finalize()
    return nc


def kernel(inputs, x0, encoders, theta, decoders, AT, Bmat):
    P, Q = _host_weights(theta, AT, Bmat, decoders, encoders)
    # qt[k, n*RANK+rho] = Q[rho, n*L + (L-1-k)]  (k-reversed within each block
    # so the device can read Hankel tiles of u with positive strides)
    qt = np.ascontiguousarray(
        Q.reshape(RANK, NCH, L)[:, :, ::-1].transpose(2, 1, 0).reshape(
            L, NCH * RANK))
    qte = np.zeros((L, QW), np.float32)
    qte[:, QPAD * RANK:] = qt
    qte = qte[KPERM]              # match the Hankel tiles' tap permutation
    qte_bf = _to_bf16(qte)
    p2 = np.zeros((2 * RANK, 2 * UNITS), np.float32)
    p2[:RANK, :UNITS] = P.T
    p2[RANK:, UNITS:] = P.T
    pt_bf = _to_bf16(np.tile(p2, (4, 1)))
    ident = np.eye(L, dtype=np.float32)

    if "nc" not in _compiled:
        _compiled["nc"] = _build_program()
    nc = _compiled["nc"]

    x = np.ascontiguousarray(np.asarray(inputs, np.float32))
    in_maps = []
    for c in range(NCORES):
        in_maps.append({
            "x": x[c * BPC:(c + 1) * BPC],
            "qte": qte_bf, "pt": pt_bf, "ident": ident,
        })
    trace = bool(os.environ.get("BASS_TRACE"))
    res = run_bass_kernel_spmd(nc, in_maps, core_ids=list(range(NCORES)),
                               trace=trace)
    _compiled["last_results"] = res
    if res.exec_time_ns is not None:
        print(f"HW exec time: {res.exec_time_ns} ns")
    y = np.concatenate([np.asarray(r["y"], np.float32) for r in res.results],
                       axis=0)
    return y


def _to_bf16(a):
    import ml_dtypes
    return np.asarray(a, np.float32).astype(ml_dtypes.bfloat16)
